# revision 1
# baseline (speedup 1.0000x reference)
"""Trainium2 Bass kernel for nn_LogicalGNNLayer (GNN message passing + MLP).

Computation (reference):
    h = term_emb[heads]; t = term_emb[tails]           # gather  [E,B,D]
    agg = segsum(s*(h+pred), tails) + segsum(s*(t+inv), heads)   # [T,B,D]
    agg += EPS*term_emb
    out = relu(agg @ W1 + b1) @ W2 + b2                # [T,B,D]

Strategy:
  - Shard batch B across 8 cores (data parallel, Bc=512 per core); the
    term/edge structure and MLP weights are replicated.
  - The gather/scatter structure depends only on the tiny heads/tails index
    arrays: read them on the host and bake the (dst, src, sign) message list
    into the kernel as a static program.
  - On-chip layout is transposed: d on partitions, (t, b) on the free axis,
    so the MLP matmuls (which contract D) consume the aggregation output
    directly with no on-device transposes.
  - Aggregation: per-term accumulators acc[k] = EPS*term[k] (DVE tensor_scalar,
    4x) then one fp16 tensor_tensor add per message operand (2x mode).
  - MLP: fp16 matmuls on PE (1 cycle/row) with fp32 PSUM accumulation;
    ReLU+bias / bias epilogues on the scalar engine straight out of PSUM.
  - fp16 on-chip halves DMA traffic (the problem is memory-bound); output is
    computed and stored in fp32.
"""

import numpy as np

import concourse.bass as bass
import concourse.tile as tile
from concourse import bacc, mybir
from concourse.bass_utils import run_bass_kernel_spmd

T, B, D, H, E = 16, 4096, 256, 512, 32
EPS = 0.1
N_CORES = 8
BC = B // N_CORES            # 512 batch per core
NB = T * BC                  # 8192 free-axis span (t, b)
DT = D // 128                # 2 d-tiles
HT = H // 128                # 4 h-tiles
NMSG = 2 * E                 # 64 directed messages
PAIR = 1024                  # MLP column chunk (2 PSUM banks)
G = 4                        # messages per streamed emb tile
F16 = mybir.dt.float16
F32 = mybir.dt.float32

_KERNEL_CACHE = {}


def _messages(heads, tails, signs):
    """Directed message list (dst, src, sign, which_emb, e), sorted by dst."""
    msgs = []
    for e in range(E):
        h, t, s = int(heads[e]), int(tails[e]), float(signs[e])
        assert 0 <= h < T and 0 <= t < T
        msgs.append((t, h, s, 0, e))   # msg_to_tail: acc[t] += s*(term[h]+pred[e])
        msgs.append((h, t, s, 1, e))   # msg_to_head: acc[h] += s*(term[t]+inv[e])
    msgs.sort(key=lambda m: m[0])
    return msgs


def _build(msgs_key, repeats=1, loop=0):
    """Build + compile the per-core SPMD Bass program for a message structure.

    repeats: statically unroll the whole body N times (timing).
    loop: wrap the body in an on-device For_i loop of N iterations (timing).
    """
    key = (msgs_key, repeats, loop)
    if key in _KERNEL_CACHE:
        return _KERNEL_CACHE[key]
    msgs = list(msgs_key)
    AF = mybir.ActivationFunctionType
    OP = mybir.AluOpType

    # groups[k] = list of (msg_idx, src, sign) with dst == k (msg_idx sorted)
    groups = [[] for _ in range(T)]
    for m, (dst, src, s, _w, _e) in enumerate(msgs):
        groups[dst].append((m, src, s))

    nc = bacc.Bacc("TRN2", target_bir_lowering=False, debug=False,
                   num_devices=N_CORES)
    termT = nc.declare_dram_parameter("termT", [D, NB], F16, isOutput=False)
    embT = nc.declare_dram_parameter("embT", [D, NMSG * BC], F16, isOutput=False)
    w1d = nc.declare_dram_parameter("w1", [D, H], F16, isOutput=False)
    w2d = nc.declare_dram_parameter("w2", [H, D], F16, isOutput=False)
    b1d = nc.declare_dram_parameter("b1t", [128, HT], F32, isOutput=False)
    b2d = nc.declare_dram_parameter("b2t", [128, DT], F32, isOutput=False)
    outT = nc.declare_dram_parameter("outT", [D, NB], F32, isOutput=True)

    with nc.allow_low_precision(reason="fp16 on-chip aggregation"), \
            tile.TileContext(nc) as tc, \
            tc.tile_pool(name="const", bufs=1) as cpool, \
            tc.tile_pool(name="term", bufs=1) as tpool, \
            tc.tile_pool(name="acc", bufs=1) as apool, \
            tc.tile_pool(name="emb", bufs=4) as epool, \
            tc.tile_pool(name="hid", bufs=8) as hpool, \
            tc.tile_pool(name="out", bufs=4) as opool, \
            tc.tile_pool(name="psum", bufs=2, space="PSUM") as pspool:

        # ---- persistent loads -------------------------------------------
        w1s = []
        w2s = []
        for dt in range(DT):
            w = cpool.tile([128, H], F16, tag=f"w1_{dt}")
            nc.sync.dma_start(w[:], w1d[dt * 128:(dt + 1) * 128, :])
            w1s.append(w)
        for ht in range(HT):
            w = cpool.tile([128, D], F16, tag=f"w2_{ht}")
            nc.sync.dma_start(w[:], w2d[ht * 128:(ht + 1) * 128, :])
            w2s.append(w)
        b1s = cpool.tile([128, HT], F32, tag="b1")
        nc.sync.dma_start(b1s[:], b1d[:])
        b2s = cpool.tile([128, DT], F32, tag="b2")
        nc.sync.dma_start(b2s[:], b2d[:])

        def body():
            terms = []
            for dt in range(DT):
                tt = tpool.tile([128, NB], F16, tag=f"term_{dt}")
                nc.sync.dma_start(tt[:], termT[dt * 128:(dt + 1) * 128, :])
                terms.append(tt)

            # ---- aggregation -------------------------------------------
            # acc[dt][k] = EPS*term_k + sum_msgs (s*term_src + s*emb_m)
            # Groups are split between DVE and GpSimd (idle otherwise) to
            # balance elementwise-add throughput; GpSimd ops are ~1.3x the
            # DVE 2x-mode cost.
            dve_t, pool_t = 0.0, 0.0
            on_pool = {}
            for k in sorted(range(T), key=lambda k: -len(groups[k])):
                c = (2 * len(groups[k]) + 1) * DT
                if pool_t + c * 1.31 < dve_t:
                    on_pool[k] = True
                    pool_t += c * 1.31
                else:
                    on_pool[k] = False
                    dve_t += c
            accs = [[None] * T for _ in range(DT)]
            for k in range(T):
                eng = nc.gpsimd if on_pool[k] else nc.vector
                for dt in range(DT):
                    a = apool.tile([128, BC], F16, tag=f"acc_{dt}_{k}")
                    accs[dt][k] = a
                    eng.tensor_scalar_mul(
                        a[:], terms[dt][:, k * BC:(k + 1) * BC], EPS)
                grp = groups[k]
                for c0 in range(0, len(grp), G):
                    chunk = grp[c0:c0 + G]
                    m0 = chunk[0][0]
                    cnt = len(chunk)
                    for dt in range(DT):
                        et = epool.tile([128, G * BC], F16, tag="emb")
                        nc.sync.dma_start(
                            et[:, :cnt * BC],
                            embT[dt * 128:(dt + 1) * 128,
                                 m0 * BC:(m0 + cnt) * BC])
                        a = accs[dt][k]
                        for i, (m, src, s) in enumerate(chunk):
                            tsl = terms[dt][:, src * BC:(src + 1) * BC]
                            if s == 1.0:
                                eng.tensor_add(a[:], a[:], tsl)
                            elif s == -1.0:
                                eng.tensor_sub(a[:], a[:], tsl)
                            else:
                                nc.vector.scalar_tensor_tensor(
                                    a[:], tsl, s, a[:], OP.mult, OP.add)
                            # emb was pre-scaled by sign on the host
                            eng.tensor_add(
                                a[:], a[:], et[:, i * BC:(i + 1) * BC])

            # ---- MLP: out = relu(agg@W1+b1)@W2 + b2 --------------------
            for p in range(NB // PAIR):
                cp = p * PAIR
                hids = []
                for ht in range(HT):
                    ps = pspool.tile([128, PAIR], F32, tag="ps1")
                    for sub in range(2):
                        k = 2 * p + sub
                        for dt in range(DT):
                            nc.tensor.matmul(
                                ps[:, sub * 512:(sub + 1) * 512],
                                w1s[dt][:, ht * 128:(ht + 1) * 128],
                                accs[dt][k][:],
                                start=(dt == 0), stop=(dt == DT - 1))
                    hid = hpool.tile([128, PAIR], F16, tag="hid")
                    nc.scalar.activation(hid[:], ps[:], AF.Relu,
                                         bias=b1s[:, ht:ht + 1], scale=1.0)
                    hids.append(hid)
                for dt2 in range(DT):
                    ps2 = pspool.tile([128, PAIR], F32, tag="ps2")
                    for sub in range(2):
                        for ht in range(HT):
                            nc.tensor.matmul(
                                ps2[:, sub * 512:(sub + 1) * 512],
                                w2s[ht][:, dt2 * 128:(dt2 + 1) * 128],
                                hids[ht][:, sub * 512:(sub + 1) * 512],
                                start=(ht == 0), stop=(ht == HT - 1))
                    ot = opool.tile([128, PAIR], F32, tag="ot")
                    nc.vector.tensor_scalar_add(ot[:], ps2[:],
                                                b2s[:, dt2:dt2 + 1])
                    nc.sync.dma_start(
                        outT[dt2 * 128:(dt2 + 1) * 128, cp:cp + PAIR], ot[:])

        if loop:
            ET = mybir.EngineType
            with tc.For_i(0, loop, 1,
                          hint_engines=(ET.PE, ET.DVE, ET.Activation, ET.SP)):
                body()
        else:
            for _rep in range(repeats):
                body()

    nc.compile()
    _KERNEL_CACHE[key] = nc
    return nc


def _prep_inputs(term_emb, pred_emb, inv_pred_emb, W1, b1, W2, b2, msgs):
    """Shard/transpose/cast host-side into the per-core device layouts."""
    t16 = term_emb.astype(np.float16)
    emb = np.empty((NMSG, B, D), np.float16)
    for m, (_dst, _src, s, which, e) in enumerate(msgs):
        arr = pred_emb if which == 0 else inv_pred_emb
        if s == 1.0:
            emb[m] = arr[e]
        else:
            emb[m] = s * arr[e]
    w1_16 = np.ascontiguousarray(W1.astype(np.float16))
    w2_16 = np.ascontiguousarray(W2.astype(np.float16))
    b1t = np.ascontiguousarray(b1.astype(np.float32).reshape(HT, 128).T)
    b2t = np.ascontiguousarray(b2.astype(np.float32).reshape(DT, 128).T)
    in_maps = []
    for c in range(N_CORES):
        sl = slice(c * BC, (c + 1) * BC)
        termTc = np.ascontiguousarray(
            t16[:, sl, :].transpose(2, 0, 1)).reshape(D, NB)
        embTc = np.ascontiguousarray(
            emb[:, sl, :].transpose(2, 0, 1)).reshape(D, NMSG * BC)
        in_maps.append(dict(termT=termTc, embT=embTc, w1=w1_16, w2=w2_16,
                            b1t=b1t, b2t=b2t))
    return in_maps


def kernel(term_emb, pred_emb, inv_pred_emb, signs, W1, b1, W2, b2,
           heads, tails):
    term_emb = np.asarray(term_emb, dtype=np.float32)
    pred_emb = np.asarray(pred_emb, dtype=np.float32)
    inv_pred_emb = np.asarray(inv_pred_emb, dtype=np.float32)
    signs = np.asarray(signs, dtype=np.float32)
    W1 = np.asarray(W1, dtype=np.float32)
    b1 = np.asarray(b1, dtype=np.float32)
    W2 = np.asarray(W2, dtype=np.float32)
    b2 = np.asarray(b2, dtype=np.float32)
    heads = np.asarray(heads).astype(np.int64)
    tails = np.asarray(tails).astype(np.int64)

    msgs = _messages(heads, tails, signs)
    nc = _build(tuple(msgs))
    in_maps = _prep_inputs(term_emb, pred_emb, inv_pred_emb, W1, b1, W2, b2,
                           msgs)
    res = run_bass_kernel_spmd(nc, in_maps, list(range(N_CORES)))

    out = np.empty((T, B, D), np.float32)
    for c in range(N_CORES):
        o = res.results[c]["outT"].reshape(D, T, BC).transpose(1, 2, 0)
        out[:, c * BC:(c + 1) * BC, :] = o
    return out



# revision 3
# speedup vs baseline: 443.6750x; 443.6750x over previous
"""Trainium2 Bass kernel for nn_LogicalGNNLayer (GNN message passing + MLP).

Computation (reference):
    h = term_emb[heads]; t = term_emb[tails]           # gather  [E,B,D]
    agg = segsum(s*(h+pred), tails) + segsum(s*(t+inv), heads)   # [T,B,D]
    agg += EPS*term_emb
    out = relu(agg @ W1 + b1) @ W2 + b2                # [T,B,D]

Strategy (v2):
  - Shard batch B across 8 cores (data parallel, Bc=512 per core); term/edge
    structure and MLP weights replicated.
  - Algebra: agg[k] = (EPS + C[k,k]) * term[k] + sum_j C[k,j]*term[j]
                    + P[k],   P[k] = sum of (sign-scaled) relation embeddings
    of messages into k.  C = signed message-count matrix (tiny, from
    heads/tails on the host; baked into the program).
  - The P[k] segment-sum never touches a compute engine: relation embeddings
    are streamed from HBM with SWDGE *accumulate* DMAs (SDMA CCE add).
    Terms are permuted into slots sorted by message count so each
    "round" (r-th message of every slot) is one contiguous accumulate DMA
    into a prefix of the accumulator tiles (each dst byte at most once per
    DMA; rounds chain via tile WAW deps, which hardware honours).
  - Term mixing is the only elementwise engine work: one tensor_scalar init
    per slot plus one op per distinct (dst,src) pair (deduped, weight
    folded), split DVE/GpSimd.
  - On-chip layout [d%128, (slot, d//128, b)]: the MLP matmuls contract D
    directly; fp16 on-chip, fp32 PSUM; ReLU+b1 on ACT; out bias + fp16 cast
    on DVE; fp16 output (upcast on host).
"""

import numpy as np

import concourse.bass as bass
import concourse.tile as tile
from concourse import bacc, mybir
from concourse.bass_utils import run_bass_kernel_spmd

T, B, D, H, E = 16, 4096, 256, 512, 32
EPS = 0.1
N_CORES = 8
BC = B // N_CORES            # 512 batch per core
DT = D // 128                # 2 d-tiles
HT = H // 128                # 4 h-tiles
NMSG = 2 * E                 # 64 directed messages
SLOT = DT * BC               # 1024 free-axis columns per term slot (dt, b)
NG = T // 2                  # 8 accumulator groups of 2 slots
NB = T * SLOT                # 16384 total free-axis span
F16 = mybir.dt.float16
F32 = mybir.dt.float32

# engine balancing: relative cost of a Pool elementwise op vs DVE, and a
# fixed Pool budget (ns) reserved for SWDGE descriptor emission.
POOL_FACTOR = 1.6
DVE_PRELOAD = 16 * 330.0     # slot inits run on DVE
OP_NS = 594.0

_KERNEL_CACHE = {}


def _messages(heads, tails, signs):
    """Directed message list (dst, src, sign, which_emb, e), sorted by dst."""
    msgs = []
    for e in range(E):
        h, t, s = int(heads[e]), int(tails[e]), float(signs[e])
        assert 0 <= h < T and 0 <= t < T
        msgs.append((t, h, s, 0, e))   # msg_to_tail: agg[t] += s*(term[h]+pred[e])
        msgs.append((h, t, s, 1, e))   # msg_to_head: agg[h] += s*(term[t]+inv[e])
    msgs.sort(key=lambda m: m[0])
    return msgs


class _Plan:
    """Static schedule derived from the message structure."""

    def __init__(self, msgs):
        per_dst = [[] for _ in range(T)]
        diag = [EPS] * T
        off = {}
        for (dstk, src, s, w, e) in msgs:
            per_dst[dstk].append((src, s, w, e))
            if src == dstk:
                diag[dstk] += s
            else:
                off[(dstk, src)] = off.get((dstk, src), 0.0) + s
        cnt = [len(per_dst[k]) for k in range(T)]
        # slots sorted by ascending message count (early groups finish their
        # accumulate chains first -> PE starts early); within each group of 2
        # descending so every round covers a prefix of the group tile.
        order = sorted(range(T), key=lambda k: (cnt[k], k))
        perm = []
        for g in range(NG):
            a, b = order[2 * g], order[2 * g + 1]
            perm += [a, b] if cnt[a] >= cnt[b] else [b, a]
        self.perm = perm
        self.slot_of = {k: s for s, k in enumerate(perm)}
        self.diag = diag
        self.cnt = cnt
        # off-diag ops per slot: (src_slot, weight)
        self.slot_ops = [[] for _ in range(T)]
        for (dstk, src), w in sorted(off.items()):
            if w != 0.0:
                self.slot_ops[self.slot_of[dstk]].append((self.slot_of[src], w))
        # emb stream (block order in embT) + accumulate rounds per group
        stream = []           # (which, e, sign)
        rounds = [[] for _ in range(NG)]   # (block_off, n_blocks)
        for g in range(NG):
            k0, k1 = perm[2 * g], perm[2 * g + 1]
            c0, c1 = cnt[k0], cnt[k1]
            for r in range(c0):
                pr = 2 if r < c1 else 1
                rounds[g].append((len(stream), pr))
                stream.append(per_dst[k0][r][2:] + (per_dst[k0][r][1],))
                if r < c1:
                    stream.append(per_dst[k1][r][2:] + (per_dst[k1][r][1],))
        # stream entries: (which, e, sign)
        self.stream = [(w, e, s) for (w, e, s) in
                       [(b[0], b[1], b[2]) for b in stream]]
        self.rounds = rounds
        self.n_accum = sum(len(r) for r in rounds)


def _assign_engines(plan):
    """Greedy slot-chain assignment to DVE / Pool balancing predicted cost."""
    costs = [(len(plan.slot_ops[s]) * OP_NS, s) for s in range(T)]
    load = {"dve": DVE_PRELOAD, "pool": plan.n_accum * 650.0}
    eng_of = {}
    for c, s in sorted(costs, reverse=True):
        if load["dve"] + c <= load["pool"] + c * POOL_FACTOR:
            eng_of[s] = "dve"
            load["dve"] += c
        else:
            eng_of[s] = "pool"
            load["pool"] += c * POOL_FACTOR
    return eng_of


def _build(msgs_key, repeats=1, loop=0):
    """Build + compile the per-core SPMD Bass program for a message structure.

    repeats: statically unroll the whole body N times (timing).
    loop: wrap the body in an on-device For_i loop of N iterations (timing).
    """
    key = (msgs_key, repeats, loop)
    if key in _KERNEL_CACHE:
        return _KERNEL_CACHE[key]
    plan = _Plan(list(msgs_key))
    AF = mybir.ActivationFunctionType
    OP = mybir.AluOpType

    nc = bacc.Bacc("TRN2", target_bir_lowering=False, debug=False,
                   num_devices=N_CORES)
    termT = nc.declare_dram_parameter("termT", [128, NB], F16, isOutput=False)
    embT = nc.declare_dram_parameter("embT", [128, NMSG * SLOT], F16,
                                     isOutput=False)
    w1d = nc.declare_dram_parameter("w1", [D, H], F16, isOutput=False)
    w2d = nc.declare_dram_parameter("w2", [H, D], F16, isOutput=False)
    b1d = nc.declare_dram_parameter("b1t", [128, HT], F32, isOutput=False)
    b2d = nc.declare_dram_parameter("b2t", [128, DT], F32, isOutput=False)
    outT = nc.declare_dram_parameter("outT", [128, NB], F16, isOutput=True)

    eng_of = _assign_engines(plan)

    with nc.allow_low_precision(reason="fp16 on-chip aggregation"), \
            tile.TileContext(nc) as tc, \
            tc.tile_pool(name="const", bufs=1) as cpool, \
            tc.tile_pool(name="term", bufs=1) as tpool, \
            tc.tile_pool(name="acc", bufs=1) as apool, \
            tc.tile_pool(name="hid", bufs=8) as hpool, \
            tc.tile_pool(name="out", bufs=4) as opool, \
            tc.tile_pool(name="psum", bufs=4, space="PSUM") as pspool:

        # ---- persistent weight loads ------------------------------------
        w1s = []
        w2s = []
        for dt in range(DT):
            w = cpool.tile([128, H], F16, tag=f"w1_{dt}")
            nc.sync.dma_start(w[:], w1d[dt * 128:(dt + 1) * 128, :])
            w1s.append(w)
        for ht in range(HT):
            w = cpool.tile([128, D], F16, tag=f"w2_{ht}")
            nc.sync.dma_start(w[:], w2d[ht * 128:(ht + 1) * 128, :])
            w2s.append(w)
        b1s = cpool.tile([128, HT], F32, tag="b1")
        nc.sync.dma_start(b1s[:], b1d[:])
        b2s = cpool.tile([128, DT], F32, tag="b2")
        nc.sync.dma_start(b2s[:], b2d[:])

        def body():
            # term tiles: 4 column chunks of 4 slots each
            terms = []
            for c in range(4):
                tt = tpool.tile([128, 4 * SLOT], F16, tag=f"term_{c}")
                nc.sync.dma_start(tt[:], termT[:, c * 4 * SLOT:(c + 1) * 4 * SLOT])
                terms.append(tt)

            def tsl(s):
                return terms[s // 4][:, (s % 4) * SLOT:(s % 4 + 1) * SLOT]

            accs = [apool.tile([128, 2 * SLOT], F16, tag=f"acc_{g}",
                               name=f"acc_{g}")
                    for g in range(NG)]

            def asl(s):
                return accs[s // 2][:, (s % 2) * SLOT:(s % 2 + 1) * SLOT]

            # ---- init: acc[slot] = (EPS + C[k,k]) * term[k] -------------
            for s in range(T):
                k = plan.perm[s]
                nc.vector.tensor_scalar_mul(asl(s), tsl(s), plan.diag[k])

            # ---- P[k]: accumulate relation embeddings via SDMA CCE ------
            for g in range(NG):
                for (boff, pr) in plan.rounds[g]:
                    nc.gpsimd.dma_start(
                        accs[g][:, :pr * SLOT],
                        embT[:, boff * SLOT:(boff + pr) * SLOT],
                        accum_op=OP.add)

            # ---- term mixing + MLP, group by group ----------------------
            for g in range(NG):
                for i in range(2):
                    s = 2 * g + i
                    eng = nc.vector if eng_of[s] == "dve" else nc.gpsimd
                    a = asl(s)
                    for (sj, w) in plan.slot_ops[s]:
                        if w == 1.0:
                            eng.tensor_add(a, a, tsl(sj))
                        elif w == -1.0:
                            eng.tensor_sub(a, a, tsl(sj))
                        else:
                            eng.scalar_tensor_tensor(a, tsl(sj), w, a,
                                                     OP.mult, OP.add)
                for i in range(2):
                    s = 2 * g + i
                    a = asl(s)
                    hids = []
                    for ht in range(HT):
                        ps = pspool.tile([128, BC], F32, tag="ps1")
                        for dt in range(DT):
                            nc.tensor.matmul(
                                ps[:],
                                w1s[dt][:, ht * 128:(ht + 1) * 128],
                                a[:, dt * BC:(dt + 1) * BC],
                                start=(dt == 0), stop=(dt == DT - 1))
                        hid = hpool.tile([128, BC], F16, tag="hid")
                        nc.scalar.activation(hid[:], ps[:], AF.Relu,
                                             bias=b1s[:, ht:ht + 1], scale=1.0)
                        hids.append(hid)
                    ot = opool.tile([128, SLOT], F16, tag="ot")
                    for dt2 in range(DT):
                        ps2 = pspool.tile([128, BC], F32, tag="ps2")
                        for ht in range(HT):
                            nc.tensor.matmul(
                                ps2[:],
                                w2s[ht][:, dt2 * 128:(dt2 + 1) * 128],
                                hids[ht][:],
                                start=(ht == 0), stop=(ht == HT - 1))
                        nc.vector.tensor_scalar_add(
                            ot[:, dt2 * BC:(dt2 + 1) * BC], ps2[:],
                            b2s[:, dt2:dt2 + 1])
                    nc.sync.dma_start(outT[:, s * SLOT:(s + 1) * SLOT], ot[:])

        if loop:
            ET = mybir.EngineType
            with tc.For_i(0, loop, 1,
                          hint_engines=(ET.PE, ET.DVE, ET.Activation, ET.SP)):
                body()
        else:
            for _rep in range(repeats):
                body()

    nc.compile()
    _KERNEL_CACHE[key] = nc
    return nc


def _prep_inputs(term_emb, pred_emb, inv_pred_emb, W1, b1, W2, b2, msgs):
    """Shard/transpose/cast host-side into the per-core device layouts."""
    plan = _Plan(msgs)
    t16 = term_emb.astype(np.float16)
    w1_16 = np.ascontiguousarray(W1.astype(np.float16))
    w2_16 = np.ascontiguousarray(W2.astype(np.float16))
    b1t = np.ascontiguousarray(b1.astype(np.float32).reshape(HT, 128).T)
    b2t = np.ascontiguousarray(b2.astype(np.float32).reshape(DT, 128).T)
    in_maps = []
    for c in range(N_CORES):
        sl = slice(c * BC, (c + 1) * BC)
        # termT[p, (slot, dt, b)] = term[perm[slot], b, dt*128+p]
        tt = t16[plan.perm, sl, :]                     # [T, BC, D]
        termTc = np.ascontiguousarray(
            tt.reshape(T, BC, DT, 128).transpose(3, 0, 2, 1)).reshape(128, NB)
        # embT blocks in stream order
        embTc = np.empty((128, NMSG * SLOT), np.float16)
        for bi, (which, e, s) in enumerate(plan.stream):
            arr = pred_emb if which == 0 else inv_pred_emb
            m = arr[e, sl, :].astype(np.float16)
            if s != 1.0:
                m = (s * m.astype(np.float32)).astype(np.float16)
            embTc[:, bi * SLOT:(bi + 1) * SLOT] = (
                m.reshape(BC, DT, 128).transpose(2, 1, 0).reshape(128, SLOT))
        in_maps.append(dict(termT=termTc, embT=embTc, w1=w1_16, w2=w2_16,
                            b1t=b1t, b2t=b2t))
    return in_maps


def _decode_out(res, msgs):
    plan = _Plan(msgs)
    out = np.empty((T, B, D), np.float32)
    for c in range(N_CORES):
        o = res.results[c]["outT"].reshape(128, T, DT, BC)
        o = o.transpose(1, 3, 2, 0).reshape(T, BC, D).astype(np.float32)
        out[plan.perm, c * BC:(c + 1) * BC, :] = o
    return out


def kernel(term_emb, pred_emb, inv_pred_emb, signs, W1, b1, W2, b2,
           heads, tails):
    term_emb = np.asarray(term_emb, dtype=np.float32)
    pred_emb = np.asarray(pred_emb, dtype=np.float32)
    inv_pred_emb = np.asarray(inv_pred_emb, dtype=np.float32)
    signs = np.asarray(signs, dtype=np.float32)
    W1 = np.asarray(W1, dtype=np.float32)
    b1 = np.asarray(b1, dtype=np.float32)
    W2 = np.asarray(W2, dtype=np.float32)
    b2 = np.asarray(b2, dtype=np.float32)
    heads = np.asarray(heads).astype(np.int64)
    tails = np.asarray(tails).astype(np.int64)

    msgs = _messages(heads, tails, signs)
    nc = _build(tuple(msgs))
    in_maps = _prep_inputs(term_emb, pred_emb, inv_pred_emb, W1, b1, W2, b2,
                           msgs)
    res = run_bass_kernel_spmd(nc, in_maps, list(range(N_CORES)))
    return _decode_out(res, msgs)


# revision 8
# speedup vs baseline: 768.6019x; 1.7324x over previous
"""Trainium2 Bass kernel for nn_LogicalGNNLayer (GNN message passing + MLP).

Computation (reference):
    h = term_emb[heads]; t = term_emb[tails]           # gather  [E,B,D]
    agg = segsum(s*(h+pred), tails) + segsum(s*(t+inv), heads)   # [T,B,D]
    agg += EPS*term_emb
    out = relu(agg @ W1 + b1) @ W2 + b2                # [T,B,D]

Strategy (v2):
  - Shard batch B across 8 cores (data parallel, Bc=512 per core); term/edge
    structure and MLP weights replicated.
  - Algebra: agg[k] = (EPS + C[k,k]) * term[k] + sum_j C[k,j]*term[j]
                    + P[k],   P[k] = sum of (sign-scaled) relation embeddings
    of messages into k.  C = signed message-count matrix (tiny, from
    heads/tails on the host; baked into the program).
  - The P[k] segment-sum never touches a compute engine: relation embeddings
    are streamed from HBM with SWDGE *accumulate* DMAs (SDMA CCE add).
    Terms are permuted into slots sorted by message count so each
    "round" (r-th message of every slot) is one contiguous accumulate DMA
    into a prefix of the accumulator tiles (each dst byte at most once per
    DMA — hardware does NOT accumulate correctly when one DMA's dst AP
    repeats bytes; rounds chain via tile WAW deps, which hardware honours).
    Rounds are emitted round-major across groups (the Pool sequencer is
    in-order, so group-major emission would serialize all chains end to
    end), and the term/acc tile rings are double-buffered so consecutive
    kernel invocations pipeline.
  - Term mixing is the only elementwise engine work: one tensor_scalar init
    per slot plus one op per distinct (dst,src) pair (deduped, weight
    folded), split DVE/GpSimd.
  - On-chip layout [d%128, (slot, d//128, b)]: the MLP matmuls contract D
    directly; fp16 on-chip, fp32 PSUM; ReLU+b1 on ACT; out bias + fp16 cast
    on DVE; fp16 output (upcast on host).
"""

import ml_dtypes
import numpy as np

import concourse.bass as bass
import concourse.tile as tile
from concourse import bacc, mybir
from concourse.bass_utils import run_bass_kernel_spmd

T, B, D, H, E = 16, 4096, 256, 512, 32
EPS = 0.1
N_CORES = 8
BC = B // N_CORES            # 512 batch per core
DT = D // 128                # 2 d-tiles
HT = H // 128                # 4 h-tiles
NMSG = 2 * E                 # 64 directed messages
SLOT = DT * BC               # 1024 free-axis columns per term slot (dt, b)
NG = T // 2                  # 8 accumulator groups of 2 slots
NB = T * SLOT                # 16384 total free-axis span
F16 = mybir.dt.float16
F32 = mybir.dt.float32
F8 = mybir.dt.float8e4

EMB_FP8 = False              # relation embeddings in fp8(e4m3): halves the
                             # dominant HBM stream; CCE casts+accumulates into
                             # fp16 accumulators (validated on HW)

# engine balancing: relative cost of a Pool elementwise op vs DVE, and a
# fixed Pool budget (ns) reserved for SWDGE descriptor emission.
POOL_FACTOR = 1.6
DVE_PRELOAD = 16 * 330.0     # slot inits run on DVE
OP_NS = 594.0

_KERNEL_CACHE = {}


def _messages(heads, tails, signs):
    """Directed message list (dst, src, sign, which_emb, e), sorted by dst."""
    msgs = []
    for e in range(E):
        h, t, s = int(heads[e]), int(tails[e]), float(signs[e])
        assert 0 <= h < T and 0 <= t < T
        msgs.append((t, h, s, 0, e))   # msg_to_tail: agg[t] += s*(term[h]+pred[e])
        msgs.append((h, t, s, 1, e))   # msg_to_head: agg[h] += s*(term[t]+inv[e])
    msgs.sort(key=lambda m: m[0])
    return msgs


class _Plan:
    """Static schedule derived from the message structure."""

    def __init__(self, msgs):
        per_dst = [[] for _ in range(T)]
        diag = [EPS] * T
        off = {}
        for (dstk, src, s, w, e) in msgs:
            per_dst[dstk].append((src, s, w, e))
            if src == dstk:
                diag[dstk] += s
            else:
                off[(dstk, src)] = off.get((dstk, src), 0.0) + s
        cnt = [len(per_dst[k]) for k in range(T)]
        # slots sorted by ascending message count (early groups finish their
        # accumulate chains first -> PE starts early); within each group of 2
        # descending so every round covers a prefix of the group tile.
        order = sorted(range(T), key=lambda k: (cnt[k], k))
        perm = []
        for g in range(NG):
            a, b = order[2 * g], order[2 * g + 1]
            perm += [a, b] if cnt[a] >= cnt[b] else [b, a]
        self.perm = perm
        self.slot_of = {k: s for s, k in enumerate(perm)}
        self.diag = diag
        self.cnt = cnt
        # off-diag ops per slot: (src_slot, weight)
        self.slot_ops = [[] for _ in range(T)]
        for (dstk, src), w in sorted(off.items()):
            if w != 0.0:
                self.slot_ops[self.slot_of[dstk]].append((self.slot_of[src], w))
        # emb stream (block order in embT) + accumulate rounds per group
        stream = []           # (which, e, sign)
        rounds = [[] for _ in range(NG)]   # (block_off, n_blocks)
        for g in range(NG):
            k0, k1 = perm[2 * g], perm[2 * g + 1]
            c0, c1 = cnt[k0], cnt[k1]
            for r in range(c0):
                pr = 2 if r < c1 else 1
                rounds[g].append((len(stream), pr))
                stream.append(per_dst[k0][r][2:] + (per_dst[k0][r][1],))
                if r < c1:
                    stream.append(per_dst[k1][r][2:] + (per_dst[k1][r][1],))
        # stream entries: (which, e, sign)
        self.stream = [(w, e, s) for (w, e, s) in
                       [(b[0], b[1], b[2]) for b in stream]]
        self.rounds = rounds
        self.n_accum = sum(len(r) for r in rounds)


def _assign_engines(plan):
    """Greedy slot-chain assignment to DVE / Pool balancing predicted cost.

    scalar_tensor_tensor is not implemented on Pool (walrus ISA check), so
    slots with any non-±1 weight are pinned to DVE.
    """
    costs = [(len(plan.slot_ops[s]) * OP_NS, s) for s in range(T)]
    load = {"dve": DVE_PRELOAD, "pool": plan.n_accum * 650.0}
    eng_of = {}
    for c, s in sorted(costs, reverse=True):
        dve_only = any(w not in (1.0, -1.0) for (_, w) in plan.slot_ops[s])
        if dve_only or load["dve"] + c <= load["pool"] + c * POOL_FACTOR:
            eng_of[s] = "dve"
            load["dve"] += c
        else:
            eng_of[s] = "pool"
            load["pool"] += c * POOL_FACTOR
    return eng_of


def _build(msgs_key, repeats=1, loop=0):
    """Build + compile the per-core SPMD Bass program for a message structure.

    repeats: statically unroll the whole body N times (timing).
    loop: wrap the body in an on-device For_i loop of N iterations (timing).
    """
    key = (msgs_key, repeats, loop, EMB_FP8)
    if key in _KERNEL_CACHE:
        return _KERNEL_CACHE[key]
    plan = _Plan(list(msgs_key))
    AF = mybir.ActivationFunctionType
    OP = mybir.AluOpType

    nc = bacc.Bacc("TRN2", target_bir_lowering=False, debug=False,
                   num_devices=N_CORES)
    termT = nc.declare_dram_parameter("termT", [128, NB], F16, isOutput=False)
    embT = nc.declare_dram_parameter("embT", [128, NMSG * SLOT],
                                     F8 if EMB_FP8 else F16, isOutput=False)
    w1d = nc.declare_dram_parameter("w1", [D, H], F16, isOutput=False)
    w2d = nc.declare_dram_parameter("w2", [H, D], F16, isOutput=False)
    b1d = nc.declare_dram_parameter("b1t", [128, HT], F32, isOutput=False)
    b2d = nc.declare_dram_parameter("b2t", [128, DT], F32, isOutput=False)
    outT = nc.declare_dram_parameter("outT", [128, NB], F16, isOutput=True)

    eng_of = _assign_engines(plan)

    with nc.allow_low_precision(reason="fp16 on-chip aggregation"), \
            tile.TileContext(nc) as tc, \
            tc.tile_pool(name="const", bufs=1) as cpool, \
            tc.tile_pool(name="term", bufs=2) as tpool, \
            tc.tile_pool(name="acc", bufs=2) as apool, \
            tc.tile_pool(name="hid", bufs=8) as hpool, \
            tc.tile_pool(name="out", bufs=4) as opool, \
            tc.tile_pool(name="psum", bufs=4, space="PSUM") as pspool:

        # ---- persistent weight loads ------------------------------------
        w1s = []
        w2s = []
        for dt in range(DT):
            w = cpool.tile([128, H], F16, tag=f"w1_{dt}")
            nc.sync.dma_start(w[:], w1d[dt * 128:(dt + 1) * 128, :])
            w1s.append(w)
        for ht in range(HT):
            w = cpool.tile([128, D], F16, tag=f"w2_{ht}")
            nc.sync.dma_start(w[:], w2d[ht * 128:(ht + 1) * 128, :])
            w2s.append(w)
        b1s = cpool.tile([128, HT], F32, tag="b1")
        nc.sync.dma_start(b1s[:], b1d[:])
        b2s = cpool.tile([128, DT], F32, tag="b2")
        nc.sync.dma_start(b2s[:], b2d[:])

        def body():
            # term tiles: 4 column chunks of 4 slots each
            terms = []
            for c in range(4):
                tt = tpool.tile([128, 4 * SLOT], F16, tag=f"term_{c}")
                nc.sync.dma_start(tt[:], termT[:, c * 4 * SLOT:(c + 1) * 4 * SLOT])
                terms.append(tt)

            def tsl(s):
                return terms[s // 4][:, (s % 4) * SLOT:(s % 4 + 1) * SLOT]

            accs = [apool.tile([128, 2 * SLOT], F16, tag=f"acc_{g}",
                               name=f"acc_{g}")
                    for g in range(NG)]

            def asl(s):
                return accs[s // 2][:, (s % 2) * SLOT:(s % 2 + 1) * SLOT]

            # ---- init: acc[slot] = (EPS + C[k,k]) * term[k] -------------
            for s in range(T):
                k = plan.perm[s]
                nc.vector.tensor_scalar_mul(asl(s), tsl(s), plan.diag[k])

            # ---- P[k]: accumulate relation embeddings via SDMA CCE ------
            # Emitted round-major: Pool's sequencer is in-order, so emitting
            # a group's whole WAW-serialized round chain back-to-back would
            # serialize ALL chains end-to-end (each round waits on the
            # previous round's completion before the next instruction can
            # even start emitting).  Interleaving rounds across groups keeps
            # ~NG accumulate DMAs in flight.
            max_rounds = max((len(r) for r in plan.rounds), default=0)
            for r in range(max_rounds):
                for g in range(NG):
                    if r < len(plan.rounds[g]):
                        boff, pr = plan.rounds[g][r]
                        nc.gpsimd.dma_start(
                            accs[g][:, :pr * SLOT],
                            embT[:, boff * SLOT:(boff + pr) * SLOT],
                            accum_op=OP.add)

            # ---- term mixing + MLP, group by group ----------------------
            for g in range(NG):
                for i in range(2):
                    s = 2 * g + i
                    eng = nc.vector if eng_of[s] == "dve" else nc.gpsimd
                    a = asl(s)
                    for (sj, w) in plan.slot_ops[s]:
                        if w == 1.0:
                            eng.tensor_add(a, a, tsl(sj))
                        elif w == -1.0:
                            eng.tensor_sub(a, a, tsl(sj))
                        else:
                            eng.scalar_tensor_tensor(a, tsl(sj), w, a,
                                                     OP.mult, OP.add)
                for i in range(2):
                    s = 2 * g + i
                    a = asl(s)
                    hids = []
                    for ht in range(HT):
                        ps = pspool.tile([128, BC], F32, tag="ps1")
                        for dt in range(DT):
                            nc.tensor.matmul(
                                ps[:],
                                w1s[dt][:, ht * 128:(ht + 1) * 128],
                                a[:, dt * BC:(dt + 1) * BC],
                                start=(dt == 0), stop=(dt == DT - 1))
                        hid = hpool.tile([128, BC], F16, tag="hid")
                        nc.scalar.activation(hid[:], ps[:], AF.Relu,
                                             bias=b1s[:, ht:ht + 1], scale=1.0)
                        hids.append(hid)
                    ot = opool.tile([128, SLOT], F16, tag="ot")
                    for dt2 in range(DT):
                        ps2 = pspool.tile([128, BC], F32, tag="ps2")
                        for ht in range(HT):
                            nc.tensor.matmul(
                                ps2[:],
                                w2s[ht][:, dt2 * 128:(dt2 + 1) * 128],
                                hids[ht][:],
                                start=(ht == 0), stop=(ht == HT - 1))
                        nc.vector.tensor_scalar_add(
                            ot[:, dt2 * BC:(dt2 + 1) * BC], ps2[:],
                            b2s[:, dt2:dt2 + 1])
                    nc.sync.dma_start(outT[:, s * SLOT:(s + 1) * SLOT], ot[:])

        if loop:
            ET = mybir.EngineType
            with tc.For_i(0, loop, 1,
                          hint_engines=(ET.PE, ET.DVE, ET.Activation, ET.SP)):
                body()
        else:
            for _rep in range(repeats):
                body()

    nc.compile()
    _KERNEL_CACHE[key] = nc
    return nc


def _prep_inputs(term_emb, pred_emb, inv_pred_emb, W1, b1, W2, b2, msgs):
    """Shard/transpose/cast host-side into the per-core device layouts."""
    plan = _Plan(msgs)
    t16 = term_emb.astype(np.float16)
    w1_16 = np.ascontiguousarray(W1.astype(np.float16))
    w2_16 = np.ascontiguousarray(W2.astype(np.float16))
    b1t = np.ascontiguousarray(b1.astype(np.float32).reshape(HT, 128).T)
    b2t = np.ascontiguousarray(b2.astype(np.float32).reshape(DT, 128).T)
    in_maps = []
    for c in range(N_CORES):
        sl = slice(c * BC, (c + 1) * BC)
        # termT[p, (slot, dt, b)] = term[perm[slot], b, dt*128+p]
        tt = t16[plan.perm, sl, :]                     # [T, BC, D]
        termTc = np.ascontiguousarray(
            tt.reshape(T, BC, DT, 128).transpose(3, 0, 2, 1)).reshape(128, NB)
        # embT blocks in stream order
        emb_np_dtype = ml_dtypes.float8_e4m3 if EMB_FP8 else np.float16
        embTc = np.empty((128, NMSG * SLOT), emb_np_dtype)
        for bi, (which, e, s) in enumerate(plan.stream):
            arr = pred_emb if which == 0 else inv_pred_emb
            m = arr[e, sl, :].astype(np.float16)
            if s != 1.0:
                m = (s * m.astype(np.float32)).astype(np.float16)
            embTc[:, bi * SLOT:(bi + 1) * SLOT] = (
                m.reshape(BC, DT, 128).transpose(2, 1, 0).reshape(128, SLOT)
                .astype(emb_np_dtype))
        in_maps.append(dict(termT=termTc, embT=embTc, w1=w1_16, w2=w2_16,
                            b1t=b1t, b2t=b2t))
    return in_maps


def _decode_out(res, msgs):
    plan = _Plan(msgs)
    out = np.empty((T, B, D), np.float32)
    for c in range(N_CORES):
        o = res.results[c]["outT"].reshape(128, T, DT, BC)
        o = o.transpose(1, 3, 2, 0).reshape(T, BC, D).astype(np.float32)
        out[plan.perm, c * BC:(c + 1) * BC, :] = o
    return out


def kernel(term_emb, pred_emb, inv_pred_emb, signs, W1, b1, W2, b2,
           heads, tails):
    term_emb = np.asarray(term_emb, dtype=np.float32)
    pred_emb = np.asarray(pred_emb, dtype=np.float32)
    inv_pred_emb = np.asarray(inv_pred_emb, dtype=np.float32)
    signs = np.asarray(signs, dtype=np.float32)
    W1 = np.asarray(W1, dtype=np.float32)
    b1 = np.asarray(b1, dtype=np.float32)
    W2 = np.asarray(W2, dtype=np.float32)
    b2 = np.asarray(b2, dtype=np.float32)
    heads = np.asarray(heads).astype(np.int64)
    tails = np.asarray(tails).astype(np.int64)

    msgs = _messages(heads, tails, signs)
    nc = _build(tuple(msgs))
    in_maps = _prep_inputs(term_emb, pred_emb, inv_pred_emb, W1, b1, W2, b2,
                           msgs)
    res = run_bass_kernel_spmd(nc, in_maps, list(range(N_CORES)))
    return _decode_out(res, msgs)
